# revision 15
# baseline (speedup 1.0000x reference)
"""Trainium2 Bass kernel for nn_AttentionBlock (B=4, C=512, T=2048, H=8, G=32).

Sharding: 8 cores = 4 batches x 2 head-groups (4 heads each).

Per core (batch b, head-group hg; local heads g=0..3):
  h   = GroupNorm32(x_b)*scale+bias -> fp8e4               (bn_stats + mask-matmuls)
  qkv = fp8 DoubleRow matmuls (157 TF/s peak; contraction C=512 as 2 insts of 128x2)
  q8/k8 folded to [32g+kk, j, t] via SBUF->SBUF DMA for K=32x2 DoubleRow score matmuls
  vT  = fp8 DoubleRow, augmented per head with 64 ones-columns so the AV matmul
        replicates the softmax denominator across PSUM partitions 64:128 for free
  ST[s-tile, tq] per head: fp8 DR (K=32x2); exp via ACT Exp->fp8 or a one-op DVE
        bit-trick (uint8(S*a+b) bits == fp8e4(e^(s-3))), split across engines
  AV  : fp8 DR over s-tile pairs -> [64 a-chans | 64 denominator rows] in PSUM
  norm: reciprocal_approx_fast on PSUM rows 64:128 -> a8 = a*rd (fp8)
  proj: fp8 DR (j-interleaved head channels; K=64x2, 2 insts per out tile)
  host: out = x + (proj_b + proj_w@v_bias) + (p0+p1)/AWO  (v-bias folded via softmax)

Scales: wq/wk x64, wv/wo x32, q/k x4 -> logits PSUM = 16*s; exp absorbs 1/16 and
a uniform e^-3 shift (cancels in softmax).
"""

import math
import os
import sys

import numpy as np
import ml_dtypes

for _p in ("/opt/trn_rl_repo", "/root/.axon_site/_ro/trn_rl_repo"):
    if _p not in sys.path and os.path.isdir(_p):
        sys.path.insert(0, _p)

B, C, T = 4, 512, 2048
H = 8
G = 32
EPS = 1e-5
CH = C // H          # 64 head dim
NCORES = 8
NKT = C // 128       # 4 contraction tiles
NTT = T // 128       # 16 sequence tiles
NTC = T // 512       # 4 t-chunks
QSCALE = 1.0 / math.sqrt(math.sqrt(CH))

AWQ = 64.0   # wq/wk weight scale
AWV = 32.0   # wv weight scale
AWO = 32.0   # wo weight scale
AQ = 4.0     # q/k activation scale
SHIFT = 3.0  # uniform logit shift (cancels in softmax)
SPS = AQ * AQ          # score psum scale (16)
LOG2E = math.log2(math.e)
EXP_A = 8.0 * LOG2E / SPS                 # DVE trick: bits = S*EXP_A + EXP_B
EXP_B = 56.0 - 8.0 * LOG2E * SHIFT - 0.5  # -0.5: corrects 1+f~2^f approx bias

# exp engine split: of the 16 s-tile exp tiles per (pair, head, tq2), indices
# in DVE_TILES go to the vector engine (bit-trick); the rest to ACT (exact).
DVE_TILES = frozenset((1, 4, 6, 9, 11, 14))

_PROG = None
LAST_RESULT = None


def _build_program():
    import concourse.bass as bass
    import concourse.tile as tile
    from concourse import mybir
    from concourse.bacc import Bacc

    F32 = mybir.dt.float32
    F8 = mybir.dt.float8e4
    U8 = mybir.dt.uint8
    AF = mybir.ActivationFunctionType
    OP = mybir.AluOpType
    DR = mybir.MatmulPerfMode.DoubleRow

    nc = Bacc(trn_type="TRN2")

    x_d = nc.dram_tensor("x", [NKT, 128, T], F32, kind="ExternalInput")
    wq_d = nc.dram_tensor("wq", [128, 2, 2, 256], F8, kind="ExternalInput")
    wk_d = nc.dram_tensor("wk", [128, 2, 2, 256], F8, kind="ExternalInput")
    wv_d = nc.dram_tensor("wv", [128, 2, 2, 256], F8, kind="ExternalInput")
    wo_d = nc.dram_tensor("wo", [64, 2, 2, 512], F8, kind="ExternalInput")
    bqk_d = nc.dram_tensor("bqk", [128, 4], F32, kind="ExternalInput")
    gm_d = nc.dram_tensor("gmask", [128, NKT, G], F32, kind="ExternalInput")
    bm_d = nc.dram_tensor("bmask", [G, NKT, 128], F32, kind="ExternalInput")
    gb_d = nc.dram_tensor("gb", [128, NKT, 2], F32, kind="ExternalInput")
    out_d = nc.dram_tensor("out", [NKT, 128, T], F32, kind="ExternalOutput")

    with tile.TileContext(nc) as tc:
        with (
            tc.tile_pool(name="singles", bufs=1) as singles,
        ):
            # ---- persistent SBUF: weights / constants ----
            wq_sb = singles.tile([128, 2, 2, 256], F8)
            nc.sync.dma_start(out=wq_sb, in_=wq_d[:, :, :, :])
            wk_sb = singles.tile([128, 2, 2, 256], F8)
            nc.sync.dma_start(out=wk_sb, in_=wk_d[:, :, :, :])
            wv_sb = singles.tile([128, 2, 2, 256], F8)
            nc.sync.dma_start(out=wv_sb, in_=wv_d[:, :, :, :])
            wo_sb = singles.tile([64, 2, 2, 512], F8)
            nc.sync.dma_start(out=wo_sb, in_=wo_d[:, :, :, :])
            bqk_sb = singles.tile([128, 4], F32)
            nc.sync.dma_start(out=bqk_sb, in_=bqk_d[:, :])
            gm_sb = singles.tile([128, NKT, G], F32)
            nc.sync.dma_start(out=gm_sb, in_=gm_d[:, :, :])
            bm_sb = singles.tile([G, NKT, 128], F32)
            nc.sync.dma_start(out=bm_sb, in_=bm_d[:, :, :])
            gb_sb = singles.tile([128, NKT, 2], F32)
            nc.sync.dma_start(out=gb_sb, in_=gb_d[:, :, :])

            ebias = singles.tile([128, 1], F32)
            nc.vector.memset(ebias, -SHIFT)
            AB = [singles.tile([128, 2], F32, name=f"ab{i}") for i in range(NKT)]
            grp2 = singles.tile([G, 2], F32)
            eps_sb = singles.tile([G, 1], F32)
            nc.vector.memset(eps_sb, EPS)

            # persistent activations
            x_sb = [singles.tile([128, T], F32, name=f"xt{i}") for i in range(NKT)]
            h8 = singles.tile([128, 2, 2, T], F8)
            qt8 = singles.tile([128, 2, T], F8)   # pre-fold q (pair-major)
            kt8 = singles.tile([128, 2, T], F8)
            q8 = singles.tile([128, 2, T], F8)    # [32g+kk, j, t]
            k8 = singles.tile([128, 2, T], F8)
            # vT augmented: [k, b, j, g, m]; m: 0:32 = ones (denominator
            # replicas land at PSUM partitions 0:32), 32:96 = v-chans
            vt8 = singles.tile([128, NTT // 2, 2, 4, 96], F8)
            nc.vector.memset(vt8[:, :, :, :, 0:32], 1.0)
            a8 = singles.tile([64, 2, 2, T], F8)  # [k, pair, j(head), t]

            for i in range(NKT):
                nc.sync.dma_start(out=x_sb[i], in_=x_d[i])

            # ================= Phase 1: GroupNorm stats =================
            with (
                tc.tile_pool(name="gnp", bufs=2) as gnp,
                tc.tile_pool(name="gps", bufs=1, space="PSUM") as gps,
            ):
                gs_ps = gps.tile([G, 2], F32, tag="gs")
                for i in range(NKT):
                    st6 = gnp.tile([128, 4, 6], F32, tag="st6")
                    for sg in range(4):
                        nc.vector.bn_stats(
                            out=st6[:, sg, :], in_=x_sb[i][:, sg * 512 : (sg + 1) * 512]
                        )
                    mv = gnp.tile([128, 2], F32, tag="mv")
                    nc.vector.bn_aggr(out=mv, in_=st6)
                    s2 = gnp.tile([128, 2], F32, tag="s2", bufs=4)
                    nc.vector.tensor_copy(out=s2[:, 0:1], in_=mv[:, 0:1])
                    nc.vector.tensor_mul(out=s2[:, 1:2], in0=mv[:, 0:1], in1=mv[:, 0:1])
                    nc.vector.tensor_add(out=s2[:, 1:2], in0=s2[:, 1:2], in1=mv[:, 1:2])
                    nc.tensor.matmul(
                        gs_ps, gm_sb[:, i, :], s2, start=(i == 0), stop=(i == NKT - 1)
                    )
                gtmp = gnp.tile([G, 2], F32, tag="gt")
                nc.vector.tensor_scalar_mul(out=gtmp, in0=gs_ps, scalar1=1.0 / 16.0)
                var = gnp.tile([G, 1], F32, tag="var")
                nc.vector.tensor_mul(out=var, in0=gtmp[:, 0:1], in1=gtmp[:, 0:1])
                nc.vector.tensor_sub(out=var, in0=gtmp[:, 1:2], in1=var)
                # rstd = exp(-0.5 * ln(var + eps))
                nc.scalar.activation(out=var, in_=var, func=AF.Ln, bias=eps_sb)
                nc.scalar.activation(out=grp2[:, 0:1], in_=var, func=AF.Exp, scale=-0.5)
                nc.vector.tensor_copy(out=grp2[:, 1:2], in_=gtmp[:, 0:1])
                for i in range(NKT):
                    ch_ps = gps.tile([128, 2], F32, tag="ch", bufs=2)
                    nc.tensor.matmul(ch_ps, bm_sb[:, i, :], grp2, start=True, stop=True)
                    # A = rstd_c * gamma ; Bc = beta - mean_c * A
                    nc.vector.tensor_mul(
                        out=AB[i][:, 0:1], in0=ch_ps[:, 0:1], in1=gb_sb[:, i, 0:1]
                    )
                    t1 = gnp.tile([128, 1], F32, tag="t1")
                    nc.vector.tensor_mul(out=t1, in0=ch_ps[:, 1:2], in1=AB[i][:, 0:1])
                    nc.vector.tensor_sub(out=AB[i][:, 1:2], in0=gb_sb[:, i, 1:2], in1=t1)
                with nc.allow_low_precision(reason="fp8 activations"):
                    for i in range(NKT):
                        eng = nc.gpsimd if i % 2 == 0 else nc.vector
                        eng.tensor_scalar(
                            out=h8[:, i // 2, i % 2, :],
                            in0=x_sb[i],
                            scalar1=AB[i][:, 0:1],
                            scalar2=AB[i][:, 1:2],
                            op0=OP.mult,
                            op1=OP.add,
                        )

            # ================= Phase 2: QKV =================
            with tc.tile_pool(name="qps", bufs=1, space="PSUM") as qps:
                with nc.allow_low_precision(reason="fp8 activations"):
                    for pair in range(2):
                        for which, w_sb, sc_col, sc in (
                            ("q", wq_sb, pair, AQ / AWQ),
                            ("k", wk_sb, 2 + pair, AQ / AWQ),
                        ):
                            dst = qt8 if which == "q" else kt8
                            qk_ps = [
                                qps.tile(
                                    [128, 512], F32, tag="qk", bufs=4,
                                    name=f"{which}ps{pair}_{tcq}",
                                )
                                for tcq in range(NTC)
                            ]
                            # i outer so each stationary load serves 4 matmuls
                            for i in range(2):
                                for tcq in range(NTC):
                                    nc.tensor.matmul(
                                        qk_ps[tcq],
                                        w_sb[:, i, :, pair * 128 : (pair + 1) * 128],
                                        h8[:, i, :, tcq * 512 : (tcq + 1) * 512],
                                        start=(i == 0),
                                        stop=(i == 1),
                                        perf_mode=DR,
                                    )
                            for tcq in range(NTC):
                                nc.vector.tensor_scalar(
                                    out=dst[:, pair, tcq * 512 : (tcq + 1) * 512],
                                    in0=qk_ps[tcq],
                                    scalar1=sc,
                                    scalar2=bqk_sb[:, sc_col : sc_col + 1],
                                    op0=OP.mult,
                                    op1=OP.add,
                                )
                        # fold this pair's q/k into [32g+kk, j, t] layout
                        for src, dst in ((qt8, q8), (kt8, k8)):
                            for ab in range(2):
                                g = 2 * pair + ab
                                for j in range(2):
                                    nc.sync.dma_start(
                                        out=dst[32 * g : 32 * g + 32, j, :],
                                        in_=src[64 * ab + 32 * j : 64 * ab + 32 * j + 32, pair, :],
                                    )
                    # vT (all 4 heads' v channels + denominator handled by ones)
                    for tt in range(NTT):
                        vt_ps = qps.tile([128, 4, 64], F32, tag="vt", bufs=2)
                        for i in range(2):
                            nc.tensor.matmul(
                                vt_ps,
                                h8[:, i, :, tt * 128 : (tt + 1) * 128],
                                wv_sb[:, i, :, :],
                                start=(i == 0),
                                stop=(i == 1),
                                perf_mode=DR,
                            )
                        nc.vector.tensor_scalar_mul(
                            out=vt8[:, tt // 2, tt % 2, :, 32:96],
                            in0=vt_ps,
                            scalar1=1.0 / AWV,
                        )

            # ================= Phase 3: attention + proj =================
            # 1024-query chunks (tq2): each stationary load (k-tile for ST,
            # v-tile for AV, wo-tile for proj) serves 2 matmuls.
            with (
                tc.tile_pool(name="sps", bufs=1, space="PSUM") as sps,
                tc.tile_pool(name="ptp", bufs=1) as ptp,
                tc.tile_pool(name="outp", bufs=1) as outp,
            ):
                for tq2 in range(2):
                    tqs = slice(tq2 * 1024, (tq2 + 1) * 1024)
                    qh_sl = [slice(tq2 * 1024 + qh * 512, tq2 * 1024 + (qh + 1) * 512)
                             for qh in range(2)]
                    for pair in range(2):
                        for ab in range(2):
                            g = 2 * pair + ab
                            gsl = slice(32 * g, 32 * g + 32)
                            aps = [
                                sps.tile([128, 512], F32, tag="aps", bufs=2,
                                         name=f"aps{g}_{tq2}_{qh}")
                                for qh in range(2)
                            ]
                            for blk in range(NTT // 2):
                                # pT layout: [s-parity, q-half, 512]
                                pT = ptp.tile([128, 2, 2, 512], F8, tag="pt",
                                              bufs=3, name="pt")
                                for sp in range(2):
                                    s2t = blk * 2 + sp
                                    ST = sps.tile([128, 2, 512], F32, tag="st",
                                                  bufs=2, name="st")
                                    for qh in range(2):
                                        nc.tensor.matmul(
                                            ST[:, qh, :],
                                            k8[gsl, :, s2t * 128 : (s2t + 1) * 128],
                                            q8[gsl, :, qh_sl[qh]],
                                            start=True,
                                            stop=True,
                                            perf_mode=DR,
                                            tile_position=(32 * g, 0),
                                        )
                                    with nc.allow_low_precision(reason="fp8 softmax"):
                                        if (2 * blk + sp) in DVE_TILES:
                                            nc.vector.tensor_scalar(
                                                out=pT[:, sp, :, :].bitcast(U8),
                                                in0=ST,
                                                scalar1=EXP_A,
                                                scalar2=EXP_B,
                                                op0=OP.mult,
                                                op1=OP.add,
                                            )
                                        else:
                                            nc.scalar.activation(
                                                out=pT[:, sp, :, :],
                                                in_=ST,
                                                func=AF.Exp,
                                                scale=1.0 / SPS,
                                                bias=ebias,
                                            )
                                for qh in range(2):
                                    nc.tensor.matmul(
                                        aps[qh][0:96, :],
                                        vt8[:, blk, :, g, :],
                                        pT[:, :, qh, :],
                                        start=(blk == 0),
                                        stop=(blk == NTT // 2 - 1),
                                        perf_mode=DR,
                                    )
                            # normalize: rows 0:32 hold the denominator
                            for qh in range(2):
                                rdb = ptp.tile([32, 512], F32, tag="rdb", bufs=2,
                                               name=f"rdb{qh}")
                                nc.vector.reciprocal_approx_fast(
                                    out=rdb, in_=aps[qh][0:32, :]
                                )
                                with nc.allow_low_precision(reason="fp8 act"):
                                    nc.vector.tensor_mul(
                                        out=a8[0:32, pair, ab, qh_sl[qh]],
                                        in0=aps[qh][32:64, :],
                                        in1=rdb,
                                    )
                                    nc.vector.tensor_mul(
                                        out=a8[32:64, pair, ab, qh_sl[qh]],
                                        in0=aps[qh][64:96, :],
                                        in1=rdb,
                                    )
                    # proj for this 1024-query chunk (all heads ready)
                    for m in range(4):
                        op_ps = sps.tile([128, 2, 512], F32, tag="fin", bufs=1,
                                         name=f"op{m}")
                        for pair in range(2):
                            for qh in range(2):
                                nc.tensor.matmul(
                                    op_ps[:, qh, :],
                                    wo_sb[:, pair, :, m * 128 : (m + 1) * 128],
                                    a8[:, pair, :, qh_sl[qh]],
                                    start=(pair == 0),
                                    stop=(pair == 1),
                                    perf_mode=DR,
                                )
                        o_sb = outp.tile([128, 2, 512], F32, tag="ot", bufs=2,
                                         name=f"osb{m}")
                        nc.vector.tensor_copy(out=o_sb, in_=op_ps)
                        nc.sync.dma_start(out=out_d[m, :, tqs], in_=o_sb)

    nc.finalize()
    return nc


def _get_program():
    global _PROG
    if _PROG is None:
        _PROG = _build_program()
    return _PROG


def _core_inputs(core, x, norm_scale, norm_bias, qkv_w, qkv_b, proj_w, proj_b):
    b, hg = core // 2, core % 2
    f32 = np.float32
    f8 = ml_dtypes.float8_e4m3fn
    hs = slice(hg * 256, hg * 256 + 256)  # head-group channel range

    def to_dr(wT, scale):  # [C, 256out] -> [128, 2, 2, 256] fp8
        return np.ascontiguousarray(
            (wT * scale).reshape(2, 2, 128, wT.shape[1]).transpose(2, 0, 1, 3)
        ).astype(f8)

    qw = qkv_w[0:C][hs]          # [256, 512]
    kw = qkv_w[C : 2 * C][hs]
    vw = qkv_w[2 * C : 3 * C][hs]
    qb = qkv_b[0:C][hs] * QSCALE * AQ
    kb = qkv_b[C : 2 * C][hs] * QSCALE * AQ

    wq = to_dr(qw.T * QSCALE, AWQ)
    wk = to_dr(kw.T * QSCALE, AWK := AWQ)
    wv = to_dr(vw.T, AWV)

    # wo: [64, 2, 2, 512]: [k, p, j, o] = proj_w[o, hs0 + 64*(2p+j) + k] * AWO
    woT = proj_w[:, hs].T * AWO  # [256, 512]
    wo = np.ascontiguousarray(
        woT.reshape(2, 2, 64, 512).transpose(2, 0, 1, 3)
    ).astype(f8)

    bqk = np.stack([qb[0:128], qb[128:256], kb[0:128], kb[128:256]], axis=1).astype(f32)

    ch_idx = np.arange(C)
    grp_of = ch_idx // 16
    gmask = np.zeros((C, G), f32)
    gmask[ch_idx, grp_of] = 1.0
    gm = np.ascontiguousarray(gmask.reshape(NKT, 128, G).transpose(1, 0, 2))
    bm = np.ascontiguousarray(gmask.T.reshape(G, NKT, 128))
    gb = np.ascontiguousarray(
        np.stack([norm_scale, norm_bias], axis=1).reshape(NKT, 128, 2).transpose(1, 0, 2)
    ).astype(f32)

    return {
        "x": np.ascontiguousarray(x[b].reshape(NKT, 128, T)).astype(f32),
        "wq": wq,
        "wk": wk,
        "wv": wv,
        "wo": wo,
        "bqk": bqk,
        "gmask": gm,
        "bmask": bm,
        "gb": gb,
    }


def kernel(x, norm_scale, norm_bias, qkv_w, qkv_b, proj_w, proj_b):
    global LAST_RESULT
    x = np.asarray(x, np.float32)
    norm_scale = np.asarray(norm_scale, np.float32)
    norm_bias = np.asarray(norm_bias, np.float32)
    qkv_w = np.asarray(qkv_w, np.float32)
    qkv_b = np.asarray(qkv_b, np.float32)
    proj_w = np.asarray(proj_w, np.float32)
    proj_b = np.asarray(proj_b, np.float32)

    from concourse.bass_utils import run_bass_kernel_spmd

    nc = _get_program()
    in_maps = [
        _core_inputs(c, x, norm_scale, norm_bias, qkv_w, qkv_b, proj_w, proj_b)
        for c in range(NCORES)
    ]
    res = run_bass_kernel_spmd(
        nc,
        in_maps,
        core_ids=list(range(NCORES)),
        trace=bool(int(os.environ.get("KERNEL_TRACE", "0"))),
    )
    LAST_RESULT = res
    # v-bias passes through softmax unchanged: fold proj_w @ vb into the bias
    vb = qkv_b[2 * C : 3 * C]
    pb_eff = proj_b + proj_w @ vb
    out = np.empty((B, C, T), np.float32)
    for b in range(B):
        p0 = res.results[2 * b]["out"].reshape(C, T)
        p1 = res.results[2 * b + 1]["out"].reshape(C, T)
        out[b] = x[b] + pb_eff[:, None] + (p0 + p1) * (1.0 / AWO)
    return out


# revision 22
# speedup vs baseline: 1.0674x; 1.0674x over previous
"""Trainium2 Bass kernel for nn_AttentionBlock (B=4, C=512, T=2048, H=8, G=32).

Sharding: 8 cores = 4 batches x 2 head-groups (4 heads each).

Per core (batch b, head-group hg; local heads g=0..3):
  h   = GroupNorm32(x_b)*scale+bias -> fp8e4               (bn_stats + mask-matmuls)
  qkv = fp8 DoubleRow matmuls (157 TF/s peak; contraction C=512 as 2 insts of 128x2)
  q8/k8 folded to [32g+kk, j, t] via SBUF->SBUF DMA for K=32x2 DoubleRow score matmuls
  vT  = fp8 DoubleRow, augmented per head with 64 ones-columns so the AV matmul
        replicates the softmax denominator across PSUM partitions 64:128 for free
  ST[s-tile, tq] per head: fp8 DR (K=32x2); exp via ACT Exp->fp8 or a one-op DVE
        bit-trick (uint8(S*a+b) bits == fp8e4(e^(s-3))), split across engines
  AV  : fp8 DR over s-tile pairs -> [64 a-chans | 64 denominator rows] in PSUM
  norm: reciprocal_approx_fast on PSUM rows 64:128 -> a8 = a*rd (fp8)
  proj: fp8 DR (j-interleaved head channels; K=64x2, 2 insts per out tile)
  host: out = x + (proj_b + proj_w@v_bias) + (p0+p1)/AWO  (v-bias folded via softmax)

Scales: wq/wk x64, wv/wo x32, q/k x4 -> logits PSUM = 16*s; exp absorbs 1/16 and
a uniform e^-3 shift (cancels in softmax).
"""

import math
import os
import sys

import numpy as np
import ml_dtypes

for _p in ("/opt/trn_rl_repo", "/root/.axon_site/_ro/trn_rl_repo"):
    if _p not in sys.path and os.path.isdir(_p):
        sys.path.insert(0, _p)

B, C, T = 4, 512, 2048
H = 8
G = 32
EPS = 1e-5
CH = C // H          # 64 head dim
NCORES = 8
NKT = C // 128       # 4 contraction tiles
NTT = T // 128       # 16 sequence tiles
NTC = T // 512       # 4 t-chunks
QSCALE = 1.0 / math.sqrt(math.sqrt(CH))

AWQ = 64.0   # wq/wk weight scale
AWV = 32.0   # wv weight scale
AWO = 32.0   # wo weight scale
AQ = 4.0     # q/k activation scale
SHIFT = 3.0  # uniform logit shift (cancels in softmax)
SPS = AQ * AQ          # score psum scale (16)
LOG2E = math.log2(math.e)
EXP_A = 8.0 * LOG2E / SPS                 # DVE trick: bits = S*EXP_A + EXP_B
EXP_B = 56.0 - 8.0 * LOG2E * SHIFT - 0.5  # -0.5: corrects 1+f~2^f approx bias

# exp engine split: of the 16 s-tile exp tiles per (pair, head, tq2), indices
# in DVE_TILES go to the vector engine (bit-trick); the rest to ACT (exact).
DVE_TILES = frozenset((3, 11))

_PROG = None
LAST_RESULT = None


def _build_program():
    import concourse.bass as bass
    import concourse.tile as tile
    from concourse import mybir
    from concourse.bacc import Bacc

    F32 = mybir.dt.float32
    F16 = mybir.dt.float16
    F8 = mybir.dt.float8e4
    U8 = mybir.dt.uint8
    AF = mybir.ActivationFunctionType
    OP = mybir.AluOpType
    DR = mybir.MatmulPerfMode.DoubleRow

    nc = Bacc(trn_type="TRN2")

    x_d = nc.dram_tensor("x", [NKT, 128, T], F32, kind="ExternalInput")
    wq_d = nc.dram_tensor("wq", [128, 2, 2, 256], F8, kind="ExternalInput")
    wk_d = nc.dram_tensor("wk", [128, 2, 2, 256], F8, kind="ExternalInput")
    wv_d = nc.dram_tensor("wv", [128, 2, 2, 256], F8, kind="ExternalInput")
    wo_d = nc.dram_tensor("wo", [64, 2, 2, 512], F8, kind="ExternalInput")
    bqk_d = nc.dram_tensor("bqk", [128, 4], F32, kind="ExternalInput")
    gm_d = nc.dram_tensor("gmask", [128, NKT, G], F32, kind="ExternalInput")
    bm_d = nc.dram_tensor("bmask", [G, NKT, 128], F32, kind="ExternalInput")
    gb_d = nc.dram_tensor("gb", [128, NKT, 2], F32, kind="ExternalInput")
    out_d = nc.dram_tensor("out", [NKT, 128, T], F32, kind="ExternalOutput")

    with tile.TileContext(nc) as tc:
        with (
            tc.tile_pool(name="singles", bufs=1) as singles,
        ):
            # ---- persistent SBUF: weights / constants ----
            wq_sb = singles.tile([128, 2, 2, 256], F8)
            nc.sync.dma_start(out=wq_sb, in_=wq_d[:, :, :, :])
            wk_sb = singles.tile([128, 2, 2, 256], F8)
            nc.sync.dma_start(out=wk_sb, in_=wk_d[:, :, :, :])
            wv_sb = singles.tile([128, 2, 2, 256], F8)
            nc.sync.dma_start(out=wv_sb, in_=wv_d[:, :, :, :])
            wo_sb = singles.tile([64, 2, 2, 512], F8)
            nc.sync.dma_start(out=wo_sb, in_=wo_d[:, :, :, :])
            bqk_sb = singles.tile([128, 4], F32)
            nc.sync.dma_start(out=bqk_sb, in_=bqk_d[:, :])
            gm_sb = singles.tile([128, NKT, G], F32)
            nc.sync.dma_start(out=gm_sb, in_=gm_d[:, :, :])
            bm_sb = singles.tile([G, NKT, 128], F32)
            nc.sync.dma_start(out=bm_sb, in_=bm_d[:, :, :])
            gb_sb = singles.tile([128, NKT, 2], F32)
            nc.sync.dma_start(out=gb_sb, in_=gb_d[:, :, :])

            ebias = singles.tile([128, 1], F32)
            nc.vector.memset(ebias, -SHIFT)
            AB = [singles.tile([128, 2], F32, name=f"ab{i}") for i in range(NKT)]
            grp2 = singles.tile([G, 2], F32)
            eps_sb = singles.tile([G, 1], F32)
            nc.vector.memset(eps_sb, EPS)

            # persistent activations
            x_sb = [singles.tile([128, T], F32, name=f"xt{i}") for i in range(NKT)]
            h8 = singles.tile([128, 2, 2, T], F8)
            q16 = singles.tile([128, 2, T], F16)  # [chan-in-pair, pair, t]
            k16 = singles.tile([128, 2, T], F16)
            # vT augmented: [k, b, j, g, m]; m: 0:32 = ones (denominator
            # replicas land at PSUM partitions 0:32), 32:96 = v-chans
            vt8 = singles.tile([128, NTT // 2, 2, 4, 96], F8)
            nc.vector.memset(vt8[:, :, :, :, 0:32], 1.0)
            a8 = singles.tile([64, 2, 2, T], F8)  # [k, pair, j(head), t]

            for i in range(NKT):
                nc.sync.dma_start(out=x_sb[i], in_=x_d[i])

            # ================= Phase 1: GroupNorm stats =================
            with (
                tc.tile_pool(name="gnp", bufs=2) as gnp,
                tc.tile_pool(name="gps", bufs=1, space="PSUM") as gps,
            ):
                gs_ps = gps.tile([G, 2], F32, tag="gs")
                for i in range(NKT):
                    st6 = gnp.tile([128, 4, 6], F32, tag="st6")
                    for sg in range(4):
                        nc.vector.bn_stats(
                            out=st6[:, sg, :], in_=x_sb[i][:, sg * 512 : (sg + 1) * 512]
                        )
                    mv = gnp.tile([128, 2], F32, tag="mv")
                    nc.vector.bn_aggr(out=mv, in_=st6)
                    s2 = gnp.tile([128, 2], F32, tag="s2", bufs=4)
                    nc.vector.tensor_copy(out=s2[:, 0:1], in_=mv[:, 0:1])
                    nc.vector.tensor_mul(out=s2[:, 1:2], in0=mv[:, 0:1], in1=mv[:, 0:1])
                    nc.vector.tensor_add(out=s2[:, 1:2], in0=s2[:, 1:2], in1=mv[:, 1:2])
                    nc.tensor.matmul(
                        gs_ps, gm_sb[:, i, :], s2, start=(i == 0), stop=(i == NKT - 1)
                    )
                gtmp = gnp.tile([G, 2], F32, tag="gt")
                nc.vector.tensor_scalar_mul(out=gtmp, in0=gs_ps, scalar1=1.0 / 16.0)
                var = gnp.tile([G, 1], F32, tag="var")
                nc.vector.tensor_mul(out=var, in0=gtmp[:, 0:1], in1=gtmp[:, 0:1])
                nc.vector.tensor_sub(out=var, in0=gtmp[:, 1:2], in1=var)
                # rstd = exp(-0.5 * ln(var + eps))
                nc.scalar.activation(out=var, in_=var, func=AF.Ln, bias=eps_sb)
                nc.scalar.activation(out=grp2[:, 0:1], in_=var, func=AF.Exp, scale=-0.5)
                nc.vector.tensor_copy(out=grp2[:, 1:2], in_=gtmp[:, 0:1])
                for i in range(NKT):
                    ch_ps = gps.tile([128, 2], F32, tag="ch", bufs=2)
                    nc.tensor.matmul(ch_ps, bm_sb[:, i, :], grp2, start=True, stop=True)
                    # A = rstd_c * gamma ; Bc = beta - mean_c * A
                    nc.vector.tensor_mul(
                        out=AB[i][:, 0:1], in0=ch_ps[:, 0:1], in1=gb_sb[:, i, 0:1]
                    )
                    t1 = gnp.tile([128, 1], F32, tag="t1")
                    nc.vector.tensor_mul(out=t1, in0=ch_ps[:, 1:2], in1=AB[i][:, 0:1])
                    nc.vector.tensor_sub(out=AB[i][:, 1:2], in0=gb_sb[:, i, 1:2], in1=t1)
                with nc.allow_low_precision(reason="fp8 activations"):
                    for i in range(NKT):
                        eng = nc.gpsimd if i % 2 == 0 else nc.vector
                        eng.tensor_scalar(
                            out=h8[:, i // 2, i % 2, :],
                            in0=x_sb[i],
                            scalar1=AB[i][:, 0:1],
                            scalar2=AB[i][:, 1:2],
                            op0=OP.mult,
                            op1=OP.add,
                        )

            # ================= Phase 2: QKV =================
            with tc.tile_pool(name="qps", bufs=1, space="PSUM") as qps:
                with nc.allow_low_precision(reason="fp8 activations"):
                    for pair in range(2):
                        for which, w_sb, sc_col, sc in (
                            ("q", wq_sb, pair, AQ / AWQ),
                            ("k", wk_sb, 2 + pair, AQ / AWQ),
                        ):
                            dst = q16 if which == "q" else k16
                            qk_ps = [
                                qps.tile(
                                    [128, 512], F32, tag="qk", bufs=4,
                                    name=f"{which}ps{pair}_{tcq}",
                                )
                                for tcq in range(NTC)
                            ]
                            # i outer so each stationary load serves 4 matmuls
                            for i in range(2):
                                for tcq in range(NTC):
                                    nc.tensor.matmul(
                                        qk_ps[tcq],
                                        w_sb[:, i, :, pair * 128 : (pair + 1) * 128],
                                        h8[:, i, :, tcq * 512 : (tcq + 1) * 512],
                                        start=(i == 0),
                                        stop=(i == 1),
                                        perf_mode=DR,
                                    )
                            for tcq in range(NTC):
                                nc.vector.tensor_scalar(
                                    out=dst[:, pair, tcq * 512 : (tcq + 1) * 512],
                                    in0=qk_ps[tcq],
                                    scalar1=sc,
                                    scalar2=bqk_sb[:, sc_col : sc_col + 1],
                                    op0=OP.mult,
                                    op1=OP.add,
                                )
                    # vT (all 4 heads' v channels + denominator handled by ones)
                    for tt in range(NTT):
                        vt_ps = qps.tile([128, 4, 64], F32, tag="vt", bufs=2)
                        for i in range(2):
                            nc.tensor.matmul(
                                vt_ps,
                                h8[:, i, :, tt * 128 : (tt + 1) * 128],
                                wv_sb[:, i, :, :],
                                start=(i == 0),
                                stop=(i == 1),
                                perf_mode=DR,
                            )
                        nc.vector.tensor_scalar_mul(
                            out=vt8[:, tt // 2, tt % 2, :, 32:96],
                            in0=vt_ps,
                            scalar1=1.0 / AWV,
                        )

            # ================= Phase 3: attention + proj =================
            # 1024-query chunks (tq2): each stationary load (k-tile for ST,
            # v-tile for AV, wo-tile for proj) serves 2 matmuls.
            with (
                tc.tile_pool(name="sps", bufs=1, space="PSUM") as sps,
                tc.tile_pool(name="ptp", bufs=1) as ptp,
                tc.tile_pool(name="outp", bufs=1) as outp,
            ):
                for tq2 in range(2):
                    tqs = slice(tq2 * 1024, (tq2 + 1) * 1024)
                    qh_sl = [slice(tq2 * 1024 + qh * 512, tq2 * 1024 + (qh + 1) * 512)
                             for qh in range(2)]
                    for pair in range(2):
                        for ab in range(2):
                            g = 2 * pair + ab
                            gsl = slice(64 * ab, 64 * ab + 64)
                            aps = [
                                sps.tile([128, 512], F32, tag="aps", bufs=2,
                                         name=f"aps{g}_{tq2}_{qh}")
                                for qh in range(2)
                            ]
                            for blk in range(NTT // 2):
                                # pT layout: [s-parity, q-half, 512]
                                pT = ptp.tile([128, 2, 2, 512], F8, tag="pt",
                                              bufs=3, name="pt")
                                for sp in range(2):
                                    s2t = blk * 2 + sp
                                    ST = sps.tile([128, 2, 512], F32, tag="st",
                                                  bufs=2, name="st")
                                    for qh in range(2):
                                        nc.tensor.matmul(
                                            ST[:, qh, :],
                                            k16[gsl, pair, s2t * 128 : (s2t + 1) * 128],
                                            q16[gsl, pair, qh_sl[qh]],
                                            start=True,
                                            stop=True,
                                        )
                                    with nc.allow_low_precision(reason="fp8 softmax"):
                                        if (2 * blk + sp) in DVE_TILES:
                                            nc.vector.tensor_scalar(
                                                out=pT[:, sp, :, :].bitcast(U8),
                                                in0=ST,
                                                scalar1=EXP_A,
                                                scalar2=EXP_B,
                                                op0=OP.mult,
                                                op1=OP.add,
                                            )
                                        else:
                                            nc.scalar.activation(
                                                out=pT[:, sp, :, :],
                                                in_=ST,
                                                func=AF.Exp,
                                                scale=1.0 / SPS,
                                                bias=ebias,
                                            )
                                for qh in range(2):
                                    nc.tensor.matmul(
                                        aps[qh][0:96, :],
                                        vt8[:, blk, :, g, :],
                                        pT[:, :, qh, :],
                                        start=(blk == 0),
                                        stop=(blk == NTT // 2 - 1),
                                        perf_mode=DR,
                                    )
                            # normalize: rows 0:32 hold the denominator
                            for qh in range(2):
                                rdb = ptp.tile([32, 512], F32, tag="rdb", bufs=2,
                                               name=f"rdb{qh}")
                                nc.vector.reciprocal_approx_fast(
                                    out=rdb, in_=aps[qh][0:32, :]
                                )
                                with nc.allow_low_precision(reason="fp8 act"):
                                    nc.vector.tensor_mul(
                                        out=a8[0:32, pair, ab, qh_sl[qh]],
                                        in0=aps[qh][32:64, :],
                                        in1=rdb,
                                    )
                                    nc.vector.tensor_mul(
                                        out=a8[32:64, pair, ab, qh_sl[qh]],
                                        in0=aps[qh][64:96, :],
                                        in1=rdb,
                                    )
                    # proj for this 1024-query chunk (all heads ready)
                    for m in range(4):
                        op_ps = sps.tile([128, 2, 512], F32, tag="fin", bufs=1,
                                         name=f"op{m}")
                        for pair in range(2):
                            for qh in range(2):
                                nc.tensor.matmul(
                                    op_ps[:, qh, :],
                                    wo_sb[:, pair, :, m * 128 : (m + 1) * 128],
                                    a8[:, pair, :, qh_sl[qh]],
                                    start=(pair == 0),
                                    stop=(pair == 1),
                                    perf_mode=DR,
                                )
                        o_sb = outp.tile([128, 2, 512], F32, tag="ot", bufs=2,
                                         name=f"osb{m}")
                        nc.vector.tensor_copy(out=o_sb, in_=op_ps)
                        nc.sync.dma_start(out=out_d[m, :, tqs], in_=o_sb)

    nc.finalize()
    return nc


def _get_program():
    global _PROG
    if _PROG is None:
        _PROG = _build_program()
    return _PROG


def _core_inputs(core, x, norm_scale, norm_bias, qkv_w, qkv_b, proj_w, proj_b):
    b, hg = core // 2, core % 2
    f32 = np.float32
    f8 = ml_dtypes.float8_e4m3fn
    hs = slice(hg * 256, hg * 256 + 256)  # head-group channel range

    def to_dr(wT, scale):  # [C, 256out] -> [128, 2, 2, 256] fp8
        return np.ascontiguousarray(
            (wT * scale).reshape(2, 2, 128, wT.shape[1]).transpose(2, 0, 1, 3)
        ).astype(f8)

    qw = qkv_w[0:C][hs]          # [256, 512]
    kw = qkv_w[C : 2 * C][hs]
    vw = qkv_w[2 * C : 3 * C][hs]
    qb = qkv_b[0:C][hs] * QSCALE * AQ
    kb = qkv_b[C : 2 * C][hs] * QSCALE * AQ

    wq = to_dr(qw.T * QSCALE, AWQ)
    wk = to_dr(kw.T * QSCALE, AWK := AWQ)
    wv = to_dr(vw.T, AWV)

    # wo: [64, 2, 2, 512]: [k, p, j, o] = proj_w[o, hs0 + 64*(2p+j) + k] * AWO
    woT = proj_w[:, hs].T * AWO  # [256, 512]
    wo = np.ascontiguousarray(
        woT.reshape(2, 2, 64, 512).transpose(2, 0, 1, 3)
    ).astype(f8)

    bqk = np.stack([qb[0:128], qb[128:256], kb[0:128], kb[128:256]], axis=1).astype(f32)

    ch_idx = np.arange(C)
    grp_of = ch_idx // 16
    gmask = np.zeros((C, G), f32)
    gmask[ch_idx, grp_of] = 1.0
    gm = np.ascontiguousarray(gmask.reshape(NKT, 128, G).transpose(1, 0, 2))
    bm = np.ascontiguousarray(gmask.T.reshape(G, NKT, 128))
    gb = np.ascontiguousarray(
        np.stack([norm_scale, norm_bias], axis=1).reshape(NKT, 128, 2).transpose(1, 0, 2)
    ).astype(f32)

    return {
        "x": np.ascontiguousarray(x[b].reshape(NKT, 128, T)).astype(f32),
        "wq": wq,
        "wk": wk,
        "wv": wv,
        "wo": wo,
        "bqk": bqk,
        "gmask": gm,
        "bmask": bm,
        "gb": gb,
    }


def kernel(x, norm_scale, norm_bias, qkv_w, qkv_b, proj_w, proj_b):
    global LAST_RESULT
    x = np.asarray(x, np.float32)
    norm_scale = np.asarray(norm_scale, np.float32)
    norm_bias = np.asarray(norm_bias, np.float32)
    qkv_w = np.asarray(qkv_w, np.float32)
    qkv_b = np.asarray(qkv_b, np.float32)
    proj_w = np.asarray(proj_w, np.float32)
    proj_b = np.asarray(proj_b, np.float32)

    from concourse.bass_utils import run_bass_kernel_spmd

    nc = _get_program()
    in_maps = [
        _core_inputs(c, x, norm_scale, norm_bias, qkv_w, qkv_b, proj_w, proj_b)
        for c in range(NCORES)
    ]
    res = run_bass_kernel_spmd(
        nc,
        in_maps,
        core_ids=list(range(NCORES)),
        trace=bool(int(os.environ.get("KERNEL_TRACE", "0"))),
    )
    LAST_RESULT = res
    # v-bias passes through softmax unchanged: fold proj_w @ vb into the bias
    vb = qkv_b[2 * C : 3 * C]
    pb_eff = proj_b + proj_w @ vb
    out = np.empty((B, C, T), np.float32)
    for b in range(B):
        p0 = res.results[2 * b]["out"].reshape(C, T)
        p1 = res.results[2 * b + 1]["out"].reshape(C, T)
        out[b] = x[b] + pb_eff[:, None] + (p0 + p1) * (1.0 / AWO)
    return out
